# revision 26
# baseline (speedup 1.0000x reference)
"""Trainium2 Bass kernel: 3-layer GraphConv GNN + MLP heads, data-parallel over batch.

Contract: kernel(**inputs) takes the FULL unsharded numpy inputs (same keys as
setup_inputs()) and returns (pi, vf) full-shape numpy arrays.

Strategy (per the data-parallel sharding hint):
  - 8 NeuronCores, 128 batch elements each. Graph structure + weights replicated.
  - The fixed graph's gather/scatter is folded on host into a dense normalized
    adjacency A [256, 256]; aggregation becomes a dense matmul on TensorE.
  - Per-core pipeline alternates two SBUF layouts:
      P: [node (2x128 partitions), (batch, feat) free]
      Q: [(batch%4, feat) = 128 partitions, (quad, node) free]
    using A(H W) = (A H) W so each layer is:
      W-GEMM (Q->P, data-stationary, moving = blockdiag4(W), bf16)
      aggregation (P->Q, data-stationary, moving = A^T, bf16)
      bias+relu fused into the PSUM->SBUF copy (ScalarE activation / VectorE
      tensor_scalar), since Q puts features on partitions.
  - All GNN-body matmuls are bf16 (PSUM accumulation stays fp32): bf16 needs no
    256-col fp32r padding, so W-GEMM moving operands shrink to 128 cols, and
    LDWEIGHTS gets the 2x FWL fast path. x/adjacency/weights are bf16; all
    intermediate h tensors are stored bf16.
  - L1 W-GEMM is weight-stationary (w1sel selection matrices): 16 matmuls of
    512 moving cols.
  - Layer-3 is never stored: the PSUM->SBUF relu is fused with the node-mean
    via accum_out (one [128,1] column sum per quad), on alternating engines.
  - Heads: emb GEMM + PE transposes to a [feat, batch] layout, then pi/vf are
    each one PSUM accumulation group of 3 matmuls (vec part, emb part, bias
    row), all fp32r with moving >= 256 cols (1 cycle/row). vector arrives
    pre-transposed [26, 128] from the host.
"""

import sys

import numpy as np

try:
    import concourse  # noqa: F401
except ImportError:  # pragma: no cover - fresh-dir fallback
    sys.path.insert(0, "/opt/trn_rl_repo")

import ml_dtypes

import concourse.bacc as bacc
import concourse.bass as bass
import concourse.mybir as mybir
import concourse.tile as tile
from concourse.bass_utils import run_bass_kernel_spmd

F32 = mybir.dt.float32
F32R = mybir.dt.float32r
BF16 = mybir.dt.bfloat16
RELU = mybir.ActivationFunctionType.Relu
ADD = mybir.AluOpType.add
MAX = mybir.AluOpType.max

N_CORES = 8
B = 1024          # total batch
NB = B // N_CORES  # batches per core = 128
N = 256           # nodes
F8 = 8            # padded input feature dim (6 -> 8)
H = 32            # hidden feature dim
VEC = 26
DPI = 512
NQ = NB // 4      # quads per core = 32
NBLK = NB // 16   # 16-batch blocks per core = 8
Q_SLOTS = 4       # psum ring depths: q + wp = 8 banks total
WP_SLOTS = 4
# evac engine = psum-ring-counter parity: a recycled psum slot's previous
# reader is always the same engine, keeping matmul sem-waits combined.
STAGGER = True    # staggered For_i sem-reset: overlaps timing-loop iterations
HINTS = (mybir.EngineType.PE, mybir.EngineType.Activation, mybir.EngineType.DVE)


def build_nc(repeat: int = 1, use_for_i: bool = False) -> bacc.Bacc:
    """Build the per-core Bass program (SPMD: all cores run this)."""
    nc = bacc.Bacc("TRN2", target_bir_lowering=False, debug=False)

    # ---- DRAM I/O ----------------------------------------------------------
    x_d = nc.dram_tensor("x", [2, 128, NB * F8], BF16, kind="ExternalInput").ap()
    vt_d = nc.dram_tensor("vt", [VEC, NB], F32R, kind="ExternalInput").ap()
    at_d = nc.dram_tensor("at", [2, 128, N], BF16, kind="ExternalInput").ap()
    w1sel_d = nc.dram_tensor("w1sel", [4, 128, 128], BF16, kind="ExternalInput").ap()
    wbd2_d = nc.dram_tensor("wbd2", [128, 128], BF16, kind="ExternalInput").ap()
    wbd3_d = nc.dram_tensor("wbd3", [128, 128], BF16, kind="ExternalInput").ap()
    wembbd_d = nc.dram_tensor("wembbd", [128, 256], F32R, kind="ExternalInput").ap()
    bembp_d = nc.dram_tensor("bembp", [1, 256], F32R, kind="ExternalInput").ap()
    biasr_d = nc.dram_tensor("biasr", [3, 128], F32, kind="ExternalInput").ap()
    ones_d = nc.dram_tensor("ones", [1, 128], F32R, kind="ExternalInput").ap()
    ident_d = nc.dram_tensor("ident", [128, 128], F32, kind="ExternalInput").ap()
    wpiv_d = nc.dram_tensor("wpiv", [VEC, DPI], F32R, kind="ExternalInput").ap()
    wpie_d = nc.dram_tensor("wpie", [H, DPI], F32R, kind="ExternalInput").ap()
    bpi_d = nc.dram_tensor("bpi", [1, DPI], F32R, kind="ExternalInput").ap()
    wvfv_d = nc.dram_tensor("wvfv", [VEC, DPI], F32R, kind="ExternalInput").ap()
    wvfe_d = nc.dram_tensor("wvfe", [H, DPI], F32R, kind="ExternalInput").ap()
    bvf_d = nc.dram_tensor("bvf", [1, DPI], F32R, kind="ExternalInput").ap()
    pi_d = nc.dram_tensor("pi", [NB, DPI], F32, kind="ExternalOutput").ap()
    vf_d = nc.dram_tensor("vf", [NB, DPI], F32, kind="ExternalOutput").ap()

    with tile.TileContext(nc) as tc:
        with (
            tc.tile_pool(name="consts", bufs=1) as cp,
            tc.tile_pool(name="acts", bufs=1) as ap_,
            tc.tile_pool(name="xp", bufs=2) as xp_,
            tc.tile_pool(name="outs", bufs=2) as op_,
            tc.tile_pool(name="scratch", bufs=3) as sp_,
            tc.tile_pool(name="l3s", bufs=4) as l3p_,
            tc.tile_pool(name="psum", bufs=1, space="PSUM") as pp_,
        ):
            # ---- constants (loaded once, outside the repeat loop) ----------
            at_sb = [cp.tile([128, N], BF16, tag=f"at{c}", name=f"at{c}") for c in range(2)]
            for c in range(2):
                nc.sync.dma_start(out=at_sb[c][:], in_=at_d[c])
            w1sel = [cp.tile([128, 128], BF16, tag=f"w1s{s}", name=f"w1s{s}") for s in range(4)]
            for s in range(4):
                nc.sync.dma_start(out=w1sel[s][:], in_=w1sel_d[s])
            wbd2 = cp.tile([128, 128], BF16, tag="wbd2")
            nc.sync.dma_start(out=wbd2[:], in_=wbd2_d[:])
            wbd3 = cp.tile([128, 128], BF16, tag="wbd3")
            nc.sync.dma_start(out=wbd3[:], in_=wbd3_d[:])
            wembbd = cp.tile([128, 256], F32R, tag="wembbd")
            nc.sync.dma_start(out=wembbd[:], in_=wembbd_d[:])
            bembp = cp.tile([1, 256], F32R, tag="bembp")
            nc.sync.dma_start(out=bembp[:], in_=bembp_d[:])
            biasr = [cp.tile([128, 1], F32, tag=f"b{l}r", name=f"b{l}r")
                     for l in range(3)]
            for l in range(3):
                nc.sync.dma_start(out=biasr[l][:], in_=biasr_d[l].unsqueeze(-1))
            ones1 = cp.tile([1, 128], F32R, tag="ones1")
            nc.sync.dma_start(out=ones1[:], in_=ones_d[:])
            ident = cp.tile([128, 128], F32, tag="ident")
            nc.sync.dma_start(out=ident[:], in_=ident_d[:])
            wpiv = cp.tile([VEC, DPI], F32R, tag="wpiv")
            nc.sync.dma_start(out=wpiv[:], in_=wpiv_d[:])
            wpie = cp.tile([H, DPI], F32R, tag="wpie")
            nc.sync.dma_start(out=wpie[:], in_=wpie_d[:])
            bpi = cp.tile([1, DPI], F32R, tag="bpi")
            nc.sync.dma_start(out=bpi[:], in_=bpi_d[:])
            wvfv = cp.tile([VEC, DPI], F32R, tag="wvfv")
            nc.sync.dma_start(out=wvfv[:], in_=wvfv_d[:])
            wvfe = cp.tile([H, DPI], F32R, tag="wvfe")
            nc.sync.dma_start(out=wvfe[:], in_=wvfe_d[:])
            bvf = cp.tile([1, DPI], F32R, tag="bvf")
            nc.sync.dma_start(out=bvf[:], in_=bvf_d[:])

            # Persistent output staging: the body DMAs out the *previous*
            # iteration's osb contents first (never blocks - that data is
            # long since written), computes, then overwrites osb.  A final
            # flush after the loop ships the last iteration's outputs.
            osb_pi = cp.tile([NB, DPI], F32, tag="opi", name="opi")
            osb_vf = cp.tile([NB, DPI], F32, tag="ovf", name="ovf")

            def body():
                # ---- load inputs --------------------------------------
                xsb = [xp_.tile([128, NB * F8], BF16, tag=f"x{c}", name=f"x{c}") for c in range(2)]
                for c in range(2):
                    nc.sync.dma_start(out=xsb[c][:], in_=x_d[c])
                vsb = xp_.tile([VEC, NB], F32R, tag="vsb")
                nc.sync.dma_start(out=vsb[:], in_=vt_d[:])
                nc.sync.dma_start(out=pi_d[:], in_=osb_pi[:])
                nc.sync.dma_start(out=vf_d[:], in_=osb_vf[:])

                # Two psum rings (agg vs W-GEMM outputs) with per-ring
                # counters; counter parity fixes the evac engine so a
                # recycled slot's previous reader matches the current
                # producer's wait chain.
                ps_n = {"q": 0, "wp": 0}

                def ps_tile(shape, name, ring):
                    i = ps_n[ring]
                    ps_n[ring] += 1
                    bufs = Q_SLOTS if ring == "q" else WP_SLOTS
                    return pp_.tile(shape, F32, tag=ring, bufs=bufs, name=name), i

                # Wait-slot discipline (see walrus S3_LW): "gate" ldweights
                # absorb producer waits on PE with no output; PSUM evac
                # engines alternate by tile parity so a recycled psum slot's
                # previous reader matches the current producer's wait chain.
                def gate(t):
                    nc.tensor.ldweights(t.bitcast(BF16))

                for c in range(2):
                    gate(xsb[c][:, 0:1])
                gate(vsb[:, 0:1])

                def relu_bias(par, dst, src, bias_ap):
                    if par % 2:
                        nc.scalar.activation(dst, src, RELU, bias=bias_ap)
                    else:
                        nc.vector.tensor_scalar(dst, src, bias_ap, 0.0, ADD, MAX)

                def plain_copy(par, dst, src):
                    if par % 2:
                        nc.scalar.copy(dst, src)
                    else:
                        nc.vector.tensor_copy(dst, src)

                # ---- L1 aggregation (P -> Q1): Z1 = (A X)^T-ish ---------
                # z1 layout: [(b16, f8)=128, (blk, n')]; 4 blks share one
                # 1024-col (two-bank) psum tile and one copy.
                z1 = ap_.tile([128, NBLK * N], BF16, tag="z1", bufs=2)
                for t in range(4):
                    q, pi_ = ps_tile([128, 2 * N], "q1", "q")
                    for half in range(2):
                        blk = 2 * t + half
                        for c in range(2):
                            nc.tensor.matmul(
                                q[:, half * N:(half + 1) * N],
                                xsb[c][:, blk * 128:(blk + 1) * 128],
                                at_sb[c][:], start=(c == 0), stop=(c == 1))
                    plain_copy(pi_, z1[:, t * 2 * N:(t + 1) * 2 * N], q[:])

                # ---- L1 W-GEMM (weight-stationary, Q1 -> Q) -------------
                # h1 = relu(Z1 W1 + b1); h1 layout: [(b4, f)=128, (g, n)]
                # with g = 4*blk + s.  For fixed s, moving = 2 blks of z1
                # (512 cols); out halves land at g = 8*hb + s, 8*hb + 4 + s.
                # Two s-values (one hb) share a two-bank psum tile.
                h1 = ap_.tile([128, NQ * N], BF16, tag="h1", bufs=2)
                h1v = h1.rearrange("p (hb k s n) -> p hb k s n", hb=4, k=2, s=4)
                for hb in range(4):
                    gate(z1[:, hb * 512:hb * 512 + 1])
                    for s in range(4):
                        q, pi_ = ps_tile([128, 512], "wq", "wp")
                        nc.tensor.matmul(
                            q[:], w1sel[s][:], z1[:, hb * 512:(hb + 1) * 512],
                            start=True, stop=True)
                        # psum col = k*256 + n; dst h1v[:, hb, k, s, n]
                        relu_bias(pi_,
                                  h1v[:, hb, :, s, :],
                                  q.rearrange("p (k n) -> p k n", k=2),
                                  biasr[0])

                # ---- L2 / L3 -------------------------------------------
                def layer(h_in, wbd, bias_ap, h_out, hg=None):
                    # W-GEMM (data-stationary Q->P): per quad g and node-half
                    # c, stationary = h_in[(b4,f), n-chunk], moving = wbd.
                    # A gq-group (4 quads) x both c-halves = 8 matmuls share
                    # one two-bank psum tile; one strided copy per tile.
                    y = sp_.tile([128, 2 * NQ * 128], BF16, tag="y", bufs=2)
                    yv = y.rearrange("p (c g m) -> p c g m", c=2, m=128)
                    y2 = y.rearrange("p (c q) -> p c q", c=2)
                    h_in_v = h_in.rearrange("p (g n) -> p g n", n=N)
                    for q4 in range(4):
                        gate(h_in[:, q4 * N:q4 * N + 1])
                    for gq in range(NQ // 4):
                        for c in range(2):
                            w, pi_ = ps_tile([128, 512], "wp", "wp")
                            for j in range(4):
                                g = gq * 4 + j
                                nc.tensor.matmul(
                                    w[:, j * 128:(j + 1) * 128],
                                    h_in_v[:, g, c * 128:(c + 1) * 128],
                                    wbd[:], start=True, stop=True)
                            plain_copy(pi_,
                                       y2[:, c, gq * 512:(gq + 1) * 512], w[:])
                    for t in range(NQ // 2):
                        q, pi_ = ps_tile([128, 2 * N], "q2", "q")
                        for half in range(2):
                            g = 2 * t + half
                            for c in range(2):
                                nc.tensor.matmul(
                                    q[:, half * N:(half + 1) * N],
                                    yv[:, c, g, :],
                                    at_sb[c][:], start=(c == 0), stop=(c == 1))
                        if h_out is not None:
                            relu_bias(pi_, h_out[:, t * 2 * N:(t + 1) * 2 * N],
                                      q[:], bias_ap)
                        else:
                            # L3: relu+bias+node-sum fused; both quads of
                            # this psum tile go to the same engine.
                            for half in range(2):
                                g = 2 * t + half
                                scr = l3p_.tile([128, N], F32R, tag="l3s")
                                if pi_ % 2:
                                    nc.scalar.activation(
                                        scr[:], q[:, half * N:(half + 1) * N],
                                        RELU, bias=bias_ap,
                                        accum_out=hg[:, g:g + 1])
                                else:
                                    nc.vector.tensor_scalar(
                                        scr[:], q[:, half * N:(half + 1) * N],
                                        bias_ap, 0.0, ADD, MAX,
                                        accum_out=hg[:, g:g + 1])

                h2 = ap_.tile([128, NQ * N], BF16, tag="h2", bufs=2)
                layer(h1, wbd2, biasr[1], h2)
                # hg is consumed by an fp32r matmul: walrus requires it be
                # *produced* as fp32r (full-precision-safe; guard silenced).
                hg = ap_.tile([128, NQ], F32R, tag="hg", bufs=2)
                with nc.allow_low_precision(reason="fp32r accum (fp32-bit-identical)"):
                    layer(h2, wbd3, biasr[2], None, hg)

                # ---- emb = (hg/256) @ W_emb + b_emb (layout [g, (b4,e)]) -
                gate(hg[:, 0:1])
                ep, ep_i = ps_tile([32, 256], "ep", "q")
                nc.tensor.matmul(ep[:], hg[:], wembbd[:],
                                 start=True, stop=False)
                nc.tensor.matmul(ep[:], ones1[:, :NQ], bembp[:],
                                 start=False, stop=True)
                embg = sp_.tile([32, 128], F32, tag="embg")
                nc.vector.tensor_copy(embg[:], ep[:, :128])

                # ---- transpose emb to [e, (g, b4)] = [32, 128] ----------
                embf = sp_.tile([32, NB], F32R, tag="embf")
                embf_v = embf.rearrange("p (g c) -> p g c", c=4)
                for b4 in range(4):
                    tp, tp_i = ps_tile([32, 32], "tp", "wp")
                    nc.tensor.matmul(tp[:], embg[:, b4 * 32:(b4 + 1) * 32],
                                     ident[:32, :32], start=True, stop=True)
                    nc.vector.tensor_copy(embf_v[:, :, b4], tp[:])

                # ---- heads ---------------------------------------------
                for wv, we, bb, out_d, tagn in (
                    (wpiv, wpie, bpi, pi_d, "pi"),
                    (wvfv, wvfe, bvf, vf_d, "vf"),
                ):
                    pp, pp_i = ps_tile([NB, DPI], f"pp{tagn}", "wp")
                    nc.tensor.matmul(pp[:], vsb[:], wv[:],
                                     start=True, stop=False)
                    nc.tensor.matmul(pp[:], embf[:], we[:],
                                     start=False, stop=False)
                    nc.tensor.matmul(pp[:], ones1[:], bb[:],
                                     start=False, stop=True)
                    osb = osb_pi if tagn == "pi" else osb_vf
                    if pp_i % 2:
                        nc.scalar.activation(osb[:], pp[:], RELU)
                    else:
                        nc.vector.tensor_scalar_max(osb[:], pp[:], 0.0)

            # one-time gates for every DMA-loaded matmul operand
            for t in (at_sb[0], at_sb[1], w1sel[0], w1sel[1], w1sel[2], w1sel[3],
                      wbd2, wbd3, wembbd, bembp, ones1, ident, wpiv, wpie, bpi,
                      wvfv, wvfe, bvf):
                nc.tensor.ldweights(t[0:1, 0:1].bitcast(BF16))

            if use_for_i and repeat > 1:
                with tc.For_i(0, repeat, 1, staggered_reset=STAGGER,
                              hint_engines=HINTS):
                    body()
            else:
                for _ in range(repeat):
                    body()

            # final flush: ship the last iteration's outputs
            nc.sync.dma_start(out=pi_d[:], in_=osb_pi[:])
            nc.sync.dma_start(out=vf_d[:], in_=osb_vf[:])

    nc.compile()
    return nc


# ---------------------------------------------------------------------------
# Host-side packing
# ---------------------------------------------------------------------------

def host_pack(inputs: dict) -> list[dict]:
    bf16 = ml_dtypes.bfloat16
    gf = np.ascontiguousarray(np.asarray(inputs["graph_feats"], dtype=np.float32))
    vec = np.ascontiguousarray(np.asarray(inputs["vector"], dtype=np.float32))
    src = np.asarray(inputs["src"]).astype(np.int64)
    dst = np.asarray(inputs["dst"]).astype(np.int64)
    W1 = np.asarray(inputs["W1"], dtype=np.float32)
    b1 = np.asarray(inputs["b1"], dtype=np.float32)
    W2 = np.asarray(inputs["W2"], dtype=np.float32)
    b2 = np.asarray(inputs["b2"], dtype=np.float32)
    W3 = np.asarray(inputs["W3"], dtype=np.float32)
    b3 = np.asarray(inputs["b3"], dtype=np.float32)
    W_emb = np.asarray(inputs["W_emb"], dtype=np.float32)
    b_emb = np.asarray(inputs["b_emb"], dtype=np.float32)
    W_pi = np.asarray(inputs["W_pi"], dtype=np.float32)
    b_pi = np.asarray(inputs["b_pi"], dtype=np.float32)
    W_vf = np.asarray(inputs["W_vf"], dtype=np.float32)
    b_vf = np.asarray(inputs["b_vf"], dtype=np.float32)

    # normalized dense adjacency (DGL GraphConv norm='both')
    deg_out = np.bincount(src, minlength=N).astype(np.float32)
    deg_in = np.bincount(dst, minlength=N).astype(np.float32)
    inv_o = np.where(deg_out > 0, deg_out ** -0.5, 0.0).astype(np.float32)
    inv_i = np.where(deg_in > 0, deg_in ** -0.5, 0.0).astype(np.float32)
    norm = inv_o[src] * inv_i[dst]
    A = np.zeros((N, N), dtype=np.float32)        # A[d, s]
    np.add.at(A, (dst, src), norm)
    AT = np.ascontiguousarray(A.T)                # AT[n, n'] = A[n', n]
    at_arr = AT.astype(bf16).reshape(2, 128, N)

    # per-core X in [nchunk, n, b*8+f] layout
    gfp = np.zeros((B, N, F8), dtype=np.float32)
    gfp[:, :, :6] = gf

    # W1 selection matrices: w1sel[s][(b16, f8), (b4, fo)] = W1[f, fo] where
    # b16 == s*4 + b4  (so quad g = 4*blk + s holds batches 16*blk + 4*s + b4)
    W1p = np.zeros((F8, H), dtype=np.float32)
    W1p[:6] = W1
    w1sel = np.zeros((4, 128, 128), dtype=np.float32)
    for s in range(4):
        for b4 in range(4):
            bb = s * 4 + b4
            w1sel[s, bb * F8:(bb + 1) * F8, b4 * H:(b4 + 1) * H] = W1p

    def blockdiag4(Wm):
        out = np.zeros((128, 128), dtype=np.float32)
        for b4 in range(4):
            out[b4 * H:(b4 + 1) * H, b4 * H:(b4 + 1) * H] = Wm
        return out

    wbd2 = blockdiag4(W2).astype(bf16)
    wbd3 = blockdiag4(W3).astype(bf16)
    w1sel = w1sel.astype(bf16)
    wembbd = np.zeros((128, 256), dtype=np.float32)
    for b4 in range(4):
        wembbd[b4 * H:(b4 + 1) * H, b4 * H:(b4 + 1) * H] = W_emb / np.float32(N)
    bembp = np.zeros((1, 256), dtype=np.float32)
    bembp[0, :128] = np.tile(b_emb, 4)
    biasr = np.stack([np.tile(b, 4) for b in (b1, b2, b3)]).astype(np.float32)
    ones = np.ones((1, 128), dtype=np.float32)
    ident = np.eye(128, dtype=np.float32)
    wpiv = np.ascontiguousarray(W_pi[:VEC])
    wpie = np.ascontiguousarray(W_pi[VEC:])
    wvfv = np.ascontiguousarray(W_vf[:VEC])
    wvfe = np.ascontiguousarray(W_vf[VEC:])
    bpi = b_pi.reshape(1, DPI)
    bvf = b_vf.reshape(1, DPI)

    in_maps = []
    for c in range(N_CORES):
        gfc = gfp[c * NB:(c + 1) * NB]                      # [128, 256, 8]
        x = np.ascontiguousarray(gfc.transpose(1, 0, 2)).reshape(N, NB * F8)
        vt = np.ascontiguousarray(vec[c * NB:(c + 1) * NB].T)  # [26, 128]
        in_maps.append({
            "x": np.ascontiguousarray(x.astype(bf16).reshape(2, 128, NB * F8)),
            "vt": vt,
            "at": at_arr, "w1sel": w1sel, "wbd2": wbd2, "wbd3": wbd3,
            "wembbd": wembbd, "bembp": bembp, "biasr": biasr, "ones": ones,
            "ident": ident, "wpiv": wpiv, "wpie": wpie, "bpi": bpi,
            "wvfv": wvfv, "wvfe": wvfe, "bvf": bvf,
        })
    return in_maps


_NC_CACHE: dict = {}


def kernel(**inputs):
    key = (1, False)
    if key not in _NC_CACHE:
        _NC_CACHE[key] = build_nc(*key)
    nc = _NC_CACHE[key]
    in_maps = host_pack(inputs)
    res = run_bass_kernel_spmd(nc, in_maps, list(range(N_CORES))).results
    pi = np.concatenate([res[c]["pi"] for c in range(N_CORES)], axis=0)
    vf = np.concatenate([res[c]["vf"] for c in range(N_CORES)], axis=0)
    return pi, vf
